# revision 10
# baseline (speedup 1.0000x reference)
"""Trainium2 Bass kernel for windowed (sparse) attention with memory KV.

Sequence-sharded across 8 NeuronCores: core c computes output tokens
[c*512, (c+1)*512) for both batches and all heads, with a 1-window (128
token) k/v halo. The full attn_bias is never shipped: only the block-
diagonal and sub-diagonal 128x128 blocks each core needs (pre-transposed,
mask folded in as -inf rows). x is shipped pre-transposed (feature-major)
so no on-device transpose is needed for the input projections.

Device dataflow (per core, per batch):
  qT = (Wq*s).T @ xT + bq*s       [1024, 512]   (feature-major, fp32r)
  kT = Wk.T @ xT                  [1024, 640]   (fp32r)
  v  = xT.T @ Wv                  [640, 1024]   (token-major, +ones col/head)
  per head pair (row-packed K=64 fp32r matmuls):
    simT chunk = kT_chunk.T @ qT  [128 keys, <=256 q]
    exp = Exp(simT + biasT)       (bias add on DVE, Exp on ACT)
    out/sumexp fused: psum[128q, 65] = exp_mem.T@mv_ext + exp_prev.T@vprev_ext
                                       + exp_cur.T@vcur_ext
    out = psum[:, :64] * recip(psum[:, 64])     (per-partition scalar)
  out_all [128q, 1024] -> PE-transpose -> y = outT.T @ Wo -> DMA out
"""

import numpy as np

B, N, DIM = 2, 4096, 768
H, DH = 16, 64
W = 128
DI = H * DH                 # 1024
NEG = -3.4028235e38
NCORES = 8
TOK = N // NCORES           # 512
NWIN = TOK // W             # 4
KTOK = TOK + W              # 640
NKC = KTOK // W             # 5
KC6 = DIM // 128            # 6 contraction chunks over DIM
DC8 = DI // 128             # 8 chunks over DI

# matmul dtype for the big projections / sim ("float32" or "float32r")
MM_DT_NAME = "float32r"


def build_bass():
    import concourse.mybir as mybir
    import concourse.tile as tile
    from concourse import bacc
    from concourse.masks import make_identity
    from contextlib import ExitStack

    f32 = mybir.dt.float32
    bf16 = mybir.dt.bfloat16
    mm_dt = getattr(mybir.dt, MM_DT_NAME)
    Exp = mybir.ActivationFunctionType.Exp
    Identity = mybir.ActivationFunctionType.Identity

    nc = bacc.Bacc("TRN2")

    # xkvT: feature-major x with halo, [B*768, 640]
    xkvT_d = nc.dram_tensor("xkvT", [B * DIM, KTOK], f32, kind="ExternalInput")
    biasc_d = nc.dram_tensor("biasc", [B * NKC * W, 2 * W], f32, kind="ExternalInput")
    wq_d = nc.dram_tensor("wq", [DIM, DI], f32, kind="ExternalInput")
    bqs_d = nc.dram_tensor("bqs", [DC8, 128], f32, kind="ExternalInput")
    wkv_d = nc.dram_tensor("wkv", [DIM, 2 * DI], f32, kind="ExternalInput")
    wo_d = nc.dram_tensor("wo", [DI, DIM], f32, kind="ExternalInput")
    memk_d = nc.dram_tensor("memk", [128, 32], f32, kind="ExternalInput")
    memv_d = nc.dram_tensor("memv", [4, 16 * 65], f32, kind="ExternalInput")
    y_d = nc.dram_tensor("y", [B * TOK, DIM], f32, kind="ExternalOutput")

    with ExitStack() as ctx:
        tc = ctx.enter_context(tile.TileContext(nc))
        # SBUF pools
        const_p = ctx.enter_context(tc.tile_pool(name="const", bufs=1))
        w_p = ctx.enter_context(tc.tile_pool(name="w", bufs=2 * KC6))
        wo_p = ctx.enter_context(tc.tile_pool(name="wo", bufs=DC8))
        xt_p = ctx.enter_context(tc.tile_pool(name="xt", bufs=KC6))
        kt_p = ctx.enter_context(tc.tile_pool(name="kt", bufs=DC8))
        qt_p = ctx.enter_context(tc.tile_pool(name="qt", bufs=DC8))
        v_p = ctx.enter_context(tc.tile_pool(name="v", bufs=NKC))
        exp_p = ctx.enter_context(tc.tile_pool(name="exp", bufs=12))
        em_p = ctx.enter_context(tc.tile_pool(name="em", bufs=4))
        oa_p = ctx.enter_context(tc.tile_pool(name="oa", bufs=NWIN))
        ot_p = ctx.enter_context(tc.tile_pool(name="ot", bufs=DC8))
        y_p = ctx.enter_context(tc.tile_pool(name="y", bufs=2))
        rc_p = ctx.enter_context(tc.tile_pool(name="rc", bufs=4))
        # single unified PSUM pool: 8 banks cycling
        ps_p = ctx.enter_context(tc.tile_pool(name="ps", bufs=8, space="PSUM"))

        def pstile(shape):
            return ps_p.tile(shape, f32, tag="ps", name="ps",
                             padded_shape=[128, 512])

        ident = const_p.tile([128, 128], f32)
        make_identity(nc, ident)

        # const tiles allocated now, DMAs issued later (off the startup path)
        bias_sb = const_p.tile([W, B * NKC * 2 * W], f32)
        memv_f32 = const_p.tile([4, 16 * 65], f32)
        memk_sb = const_p.tile([128, 32], mm_dt)
        memv_sb = const_p.tile([4, 16 * 65], bf16)
        bqs_sb = const_p.tile([128, DC8], f32)
        wo_sb = [wo_p.tile([128, DIM], mm_dt, tag="wo", name=f"wo{_}")
                 for _ in range(DC8)]

        def load_consts():
            for bb in range(B):
                for kc in range(NKC):
                    col = (bb * NKC + kc) * 2 * W
                    nc.gpsimd.dma_start(
                        bias_sb[:, col:col + 2 * W],
                        biasc_d[(bb * NKC + kc) * W:(bb * NKC + kc + 1) * W, :])
            nc.gpsimd.dma_start(memk_sb, memk_d[:, :].bitcast(mm_dt))
            nc.gpsimd.dma_start(memv_f32, memv_d[:, :])
            nc.vector.tensor_copy(memv_sb, memv_f32)
            nc.gpsimd.dma_start(bqs_sb, bqs_d.rearrange("c p -> p c"))
            for d in range(DC8):
                nc.gpsimd.dma_start(
                    wo_sb[d], wo_d[d * 128:(d + 1) * 128, :].bitcast(mm_dt))

        for b in range(B):
            # ---- xT load (pre-transposed on host) ----
            xT = [xt_p.tile([128, KTOK], mm_dt, tag="xt", name=f"xt{_}")
                  for _ in range(KC6)]
            for d in range(KC6):
                r0 = b * DIM + d * 128
                nc.sync.dma_start(xT[d], xkvT_d[r0:r0 + 128, :].bitcast(mm_dt))

            # ---- kT = Wk.T @ xT ----
            wk = [w_p.tile([128, DI], mm_dt, tag="w", name=f"w{_}")
                  for _ in range(KC6)]
            for d in range(KC6):
                nc.sync.dma_start(
                    wk[d], wkv_d[d * 128:(d + 1) * 128, :DI].bitcast(mm_dt))
            kT = [kt_p.tile([128, KTOK], mm_dt, tag="kt", name=f"kt{_}")
                  for _ in range(DC8)]
            for d8 in range(DC8):
                for nt in range(2):
                    ps = pstile([128, 320])
                    for k6 in range(KC6):
                        nc.tensor.matmul(
                            ps, wk[k6][:, d8 * 128:(d8 + 1) * 128],
                            xT[k6][:, nt * 320:(nt + 1) * 320],
                            start=(k6 == 0), stop=(k6 == KC6 - 1))
                    nc.scalar.copy(kT[d8][:, nt * 320:(nt + 1) * 320], ps)

            if b == 0:
                load_consts()

            # ---- qT = (Wq*s).T @ xT + bq*s ----
            wqs = [w_p.tile([128, DI], mm_dt, tag="w", name=f"w{_}")
                   for _ in range(KC6)]
            for d in range(KC6):
                nc.sync.dma_start(
                    wqs[d], wq_d[d * 128:(d + 1) * 128, :].bitcast(mm_dt))
            qT = [qt_p.tile([128, TOK], mm_dt, tag="qt", name=f"qt{_}")
                  for _ in range(DC8)]
            for d8 in range(DC8):
                ps = pstile([128, 512])
                for k6 in range(KC6):
                    nc.tensor.matmul(
                        ps, wqs[k6][:, d8 * 128:(d8 + 1) * 128],
                        xT[k6][:, W:W + TOK],
                        start=(k6 == 0), stop=(k6 == KC6 - 1))
                nc.scalar.activation(qT[d8], ps, Identity,
                                     bias=bqs_sb[:, d8:d8 + 1])

            # ---- v = xT.T @ Wv (token-major, 65-strided + ones col) ----
            wv = [w_p.tile([128, DI], mm_dt, tag="w", name=f"w{_}")
                  for _ in range(KC6)]
            for d in range(KC6):
                nc.sync.dma_start(
                    wv[d], wkv_d[d * 128:(d + 1) * 128, DI:].bitcast(mm_dt))
            v_ext = [v_p.tile([128, 16 * 65], bf16, tag="v", name=f"v{_}")
                     for _ in range(NKC)]
            for tt in range(NKC):
                v3 = v_ext[tt].rearrange("p (h c) -> p h c", c=65)
                nc.vector.memset(v3[:, :, 64:65], 1.0)
                for half in range(2):
                    ps = pstile([128, 512])
                    for k6 in range(KC6):
                        nc.tensor.matmul(
                            ps, xT[k6][:, tt * 128:(tt + 1) * 128],
                            wv[k6][:, half * 512:(half + 1) * 512],
                            start=(k6 == 0), stop=(k6 == KC6 - 1))
                    nc.vector.tensor_copy(
                        v3[:, half * 8:(half + 1) * 8, 0:64],
                        ps.rearrange("p (h c) -> p h c", c=64))

            # ---- attention ----
            out_all = [oa_p.tile([128, DI], f32, tag="oa", name=f"oa{_}")
                       for _ in range(NWIN)]
            for hp in range(DC8):
                emem = []
                for h01 in range(2):
                    rows = slice(64 * h01, 64 * h01 + 64)
                    psm = pstile([128, 512])[:4]
                    nc.tensor.matmul(
                        psm, memk_sb[rows, hp * 4:(hp + 1) * 4],
                        qT[hp][rows, :], start=True, stop=True)
                    et = em_p.tile([4, 512], bf16, tag="em", name="em")
                    nc.scalar.activation(et, psm, Exp)
                    emem.append(et)
                exp_tiles = {}
                for kc in range(NKC):
                    qlo = max(0, (kc - 1) * W)
                    qhi = min(TOK, (kc + 1) * W)
                    qw = qhi - qlo
                    off = qlo - (kc - 1) * W
                    for h01 in range(2):
                        rows = slice(64 * h01, 64 * h01 + 64)
                        ps = pstile([128, 256])[:, :qw]
                        nc.tensor.matmul(
                            ps, kT[hp][rows, kc * W:(kc + 1) * W],
                            qT[hp][rows, qlo:qhi], start=True, stop=True)
                        et = exp_p.tile([128, 256], f32, tag="exp",
                                        name="exp")[:, :qw]
                        eb = exp_p.tile([128, 256], bf16, tag="expb",
                                        name="expb")[:, :qw]
                        bcol = (b * NKC + kc) * 2 * W + off
                        nc.vector.tensor_add(et, ps, bias_sb[:, bcol:bcol + qw])
                        nc.scalar.activation(eb, et, Exp)
                        exp_tiles[(h01, kc)] = eb
                for w in range(NWIN):
                    for h01 in range(2):
                        hg = 2 * hp + h01
                        pcol = 0 if w == 0 else W
                        prev_e = exp_tiles[(h01, w)]
                        cur_e = exp_tiles[(h01, w + 1)]
                        psv = pstile([128, 65])
                        nc.tensor.matmul(
                            psv, emem[h01][:, w * W:(w + 1) * W],
                            memv_sb[:, hg * 65:(hg + 1) * 65],
                            start=True, stop=False)
                        nc.tensor.matmul(
                            psv, prev_e[:, pcol:pcol + W],
                            v_ext[w].rearrange(
                                "p (h c) -> p h c", c=65)[:, hg],
                            start=False, stop=False)
                        nc.tensor.matmul(
                            psv, cur_e[:, 0:W],
                            v_ext[w + 1].rearrange(
                                "p (h c) -> p h c", c=65)[:, hg],
                            start=False, stop=True)
                        rc = rc_p.tile([128, 1], f32, tag="rc", name="rc")
                        nc.vector.reciprocal(rc, psv[:, 64:65])
                        nc.vector.tensor_scalar_mul(
                            out_all[w][:, hg * 64:(hg + 1) * 64],
                            psv[:, 0:64], rc)

            # ---- out transpose + final projection ----
            for w in range(NWIN):
                outT = [ot_p.tile([128, 128], mm_dt, tag="ot", name=f"ot{_}")
                        for _ in range(DC8)]
                for d8 in range(DC8):
                    ps = pstile([128, 128])
                    nc.tensor.transpose(
                        ps, out_all[w][:, d8 * 128:(d8 + 1) * 128], ident)
                    if d8 % 2 == 0:
                        nc.vector.tensor_copy(outT[d8], ps)
                    else:
                        nc.scalar.copy(outT[d8], ps)
                ysb = y_p.tile([128, DIM], f32, tag="y", name="y")
                for nn in range(2):
                    ps = pstile([128, 384])
                    for d8 in range(DC8):
                        nc.tensor.matmul(
                            ps, outT[d8], wo_sb[d8][:, nn * 384:(nn + 1) * 384],
                            start=(d8 == 0), stop=(d8 == DC8 - 1))
                    if nn == 0:
                        nc.vector.tensor_copy(ysb[:, nn * 384:(nn + 1) * 384], ps)
                    else:
                        nc.scalar.copy(ysb[:, nn * 384:(nn + 1) * 384], ps)
                nc.sync.dma_start(
                    y_d[b * TOK + w * W:b * TOK + (w + 1) * W, :], ysb)
    nc.compile()
    return nc


def host_prep(x, mask, attn_bias, Wq, bq, Wkv, Wo, memory_kv):
    s = np.float32(DH ** -0.5)
    wq = (np.asarray(Wq, np.float32) * s).astype(np.float32)
    bqs = (np.asarray(bq, np.float32) * s).astype(np.float32).reshape(DC8, 128)
    wkv = np.ascontiguousarray(np.asarray(Wkv, np.float32))
    wo = np.ascontiguousarray(np.asarray(Wo, np.float32))
    x = np.asarray(x, np.float32)
    mask = np.asarray(mask).astype(bool)
    attn_bias = np.asarray(attn_bias, np.float32)
    mk = np.asarray(memory_kv[0], np.float32)
    mv = np.asarray(memory_kv[1], np.float32)

    memk = np.zeros((128, 32), np.float32)
    for hp in range(8):
        memk[0:64, hp * 4:(hp + 1) * 4] = mk[2 * hp].T
        memk[64:128, hp * 4:(hp + 1) * 4] = mk[2 * hp + 1].T
    memv = np.zeros((4, 16 * 65), np.float32)
    for h in range(H):
        memv[:, h * 65:h * 65 + 64] = mv[h]
        memv[:, h * 65 + 64] = 1.0

    shared = dict(wq=wq, bqs=bqs, wkv=wkv, wo=wo, memk=memk, memv=memv)
    xT_full = np.ascontiguousarray(x.transpose(0, 2, 1))    # [B, 768, 4096]
    in_maps = []
    for c in range(NCORES):
        q0 = c * TOK
        xkvT = np.zeros((B, DIM, KTOK), np.float32)
        lo = q0 - W
        src_lo = max(lo, 0)
        xkvT[:, :, src_lo - lo:] = xT_full[:, :, src_lo:q0 + TOK]
        biasc = np.full((B, NKC, W, 2 * W), NEG, np.float32)
        for b in range(B):
            for kc in range(NKC):
                gk = c * NWIN + kc - 1
                if gk < 0:
                    continue
                kr = slice(gk * W, (gk + 1) * W)
                if kc >= 1:
                    qr = slice((c * NWIN + kc - 1) * W, (c * NWIN + kc) * W)
                    biasc[b, kc, :, 0:W] = attn_bias[b, qr, kr].T
                if kc <= NWIN - 1:
                    qr = slice((c * NWIN + kc) * W, (c * NWIN + kc + 1) * W)
                    biasc[b, kc, :, W:2 * W] = attn_bias[b, qr, kr].T
                kmask = mask[b, gk * W:(gk + 1) * W]
                biasc[b, kc, ~kmask, :] = NEG
        in_maps.append(dict(
            xkvT=np.ascontiguousarray(xkvT.reshape(B * DIM, KTOK)),
            biasc=np.ascontiguousarray(biasc.reshape(B * NKC * W, 2 * W)),
            **shared))
    return in_maps


_CACHE = {}


def kernel(**inputs):
    import sys
    if "/opt/trn_rl_repo" not in sys.path:
        sys.path.insert(0, "/opt/trn_rl_repo")
    from concourse.bass_utils import run_bass_kernel_spmd

    in_maps = host_prep(**inputs)
    if "nc" not in _CACHE:
        _CACHE["nc"] = build_bass()
    nc = _CACHE["nc"]
    res = run_bass_kernel_spmd(nc, in_maps, core_ids=list(range(NCORES)))
    ys = [res.results[c]["y"].reshape(B, TOK, DIM) for c in range(NCORES)]
    return np.concatenate(ys, axis=1)


if __name__ == "__main__":
    import sys
    sys.path.insert(0, "/opt/trn_rl_repo")
    nc = build_bass()
    print("build OK")


# revision 22
# speedup vs baseline: 1.1004x; 1.1004x over previous
"""Trainium2 Bass kernel for windowed (sparse) attention with memory KV.

Sequence-sharded across 8 NeuronCores: core c computes output tokens
[c*512, (c+1)*512) for both batches and all heads, with a 1-window (128
token) k/v halo. The full attn_bias is never shipped: only the block-
diagonal and sub-diagonal 128x128 blocks each core needs (pre-transposed,
mask folded in as -inf rows). x is shipped pre-transposed (feature-major)
so no on-device transpose is needed for the input projections.

Device dataflow (per core, per batch):
  qT = (Wq*s).T @ xT + bq*s       [1024, 512]   (feature-major, fp32r)
  kT = Wk.T @ xT                  [1024, 640]   (fp32r)
  v  = xT.T @ Wv                  [640, 1024]   (token-major, +ones col/head)
  per head pair (row-packed K=64 fp32r matmuls):
    simT chunk = kT_chunk.T @ qT  [128 keys, <=256 q]
    exp = Exp(simT + biasT)       (bias add on DVE, Exp on ACT)
    out/sumexp fused: psum[128q, 65] = exp_mem.T@mv_ext + exp_prev.T@vprev_ext
                                       + exp_cur.T@vcur_ext
    out = psum[:, :64] * recip(psum[:, 64])     (per-partition scalar)
  out_all [128q, 1024] -> PE-transpose -> y = outT.T @ Wo -> DMA out
"""

import numpy as np

B, N, DIM = 2, 4096, 768
H, DH = 16, 64
W = 128
DI = H * DH                 # 1024
NEG = -3.4028235e38
BNEG = -1.0e30          # masked-bias value: exp() underflows to 0, but stays
                        # finite under fp32r rounding (0 * -inf would be NaN)
NCORES = 8
TOK = N // NCORES           # 512
NWIN = TOK // W             # 4
KTOK = TOK + W              # 640
NKC = KTOK // W             # 5
KC6 = DIM // 128            # 6 contraction chunks over DIM
DC8 = DI // 128             # 8 chunks over DI

# matmul dtype for the big projections / sim ("float32" or "float32r")
MM_DT_NAME = "float32r"


def build_bass():
    import concourse.mybir as mybir
    import concourse.tile as tile
    from concourse import bacc
    from concourse.masks import make_identity
    from contextlib import ExitStack

    f32 = mybir.dt.float32
    bf16 = mybir.dt.bfloat16
    mm_dt = getattr(mybir.dt, MM_DT_NAME)
    Exp = mybir.ActivationFunctionType.Exp
    Identity = mybir.ActivationFunctionType.Identity

    nc = bacc.Bacc("TRN2")

    # xkvT: feature-major x with halo, [B*768, 640]
    xkvT_d = nc.dram_tensor("xkvT", [B * DIM, KTOK], f32, kind="ExternalInput")
    biasc_d = nc.dram_tensor("biasc", [B * NKC * W, 2 * W], f32, kind="ExternalInput")
    wq_d = nc.dram_tensor("wq", [DIM, DI], f32, kind="ExternalInput")
    bqs_d = nc.dram_tensor("bqs", [DC8, 128], f32, kind="ExternalInput")
    wkv_d = nc.dram_tensor("wkv", [DIM, 2 * DI], f32, kind="ExternalInput")
    wo_d = nc.dram_tensor("wo", [DI, DIM], f32, kind="ExternalInput")
    memk_d = nc.dram_tensor("memk", [128, 32], f32, kind="ExternalInput")
    memv_d = nc.dram_tensor("memv", [4, 16 * 65], bf16, kind="ExternalInput")
    y_d = nc.dram_tensor("y", [B * TOK, DIM], f32, kind="ExternalOutput")

    with ExitStack() as ctx:
        tc = ctx.enter_context(tile.TileContext(nc))
        # SBUF pools
        const_p = ctx.enter_context(tc.tile_pool(name="const", bufs=1))
        w_p = ctx.enter_context(tc.tile_pool(name="w", bufs=3 * KC6))
        wo_p = ctx.enter_context(tc.tile_pool(name="wo", bufs=DC8))
        xt_p = ctx.enter_context(tc.tile_pool(name="xt", bufs=8))
        kt_p = ctx.enter_context(tc.tile_pool(name="kt", bufs=DC8))
        qt_p = ctx.enter_context(tc.tile_pool(name="qt", bufs=DC8))
        v_p = ctx.enter_context(tc.tile_pool(name="v", bufs=NKC))
        exp_p = ctx.enter_context(tc.tile_pool(name="exp", bufs=12))
        em_p = ctx.enter_context(tc.tile_pool(name="em", bufs=3))
        ot_p = ctx.enter_context(tc.tile_pool(name="ot", bufs=DC8))
        y_p = ctx.enter_context(tc.tile_pool(name="y", bufs=1))
        rc_p = ctx.enter_context(tc.tile_pool(name="rc", bufs=4))
        # single unified PSUM pool: 8 banks cycling
        ps_p = ctx.enter_context(tc.tile_pool(name="ps", bufs=8, space="PSUM"))

        def pstile(shape):
            return ps_p.tile(shape, f32, tag="ps", name="ps",
                             padded_shape=[128, 512])

        ident = const_p.tile([128, 128], f32)
        make_identity(nc, ident)
        identr = const_p.tile([128, 128], mm_dt)
        nc.vector.tensor_copy(identr, ident)

        # const tiles allocated now, DMAs issued later (off the startup path)
        bias_sb = const_p.tile([W, B * NKC * 2 * W], mm_dt)
        memk_sb = const_p.tile([128, 32], mm_dt)
        memv_sb = const_p.tile([4, 16 * 65], bf16)
        bqs_sb = const_p.tile([128, DC8], f32)
        wo_sb = [wo_p.tile([128, DIM], mm_dt, tag="wo", name=f"wo{_}")
                 for _ in range(DC8)]

        def load_consts():
            for bb in range(B):
                for kc in range(NKC):
                    col = (bb * NKC + kc) * 2 * W
                    nc.gpsimd.dma_start(
                        bias_sb[:, col:col + 2 * W],
                        biasc_d[(bb * NKC + kc) * W:(bb * NKC + kc + 1) * W,
                                :].bitcast(mm_dt))
            nc.gpsimd.dma_start(memk_sb, memk_d[:, :].bitcast(mm_dt))
            nc.gpsimd.dma_start(memv_sb, memv_d[:, :])
            for d in range(DC8):
                nc.gpsimd.dma_start(
                    wo_sb[d], wo_d[d * 128:(d + 1) * 128, :].bitcast(mm_dt))

        nc.gpsimd.dma_start(bqs_sb, bqs_d.rearrange("c p -> p c"))
        wcache = {}
        for b in range(B):
            # ---- xT load (pre-transposed on host) ----
            xT = [xt_p.tile([128, 1024], mm_dt, tag="xtoa", name=f"xt{_}",
                            bufs=8)[:, :KTOK] for _ in range(KC6)]
            for d in range(KC6):
                r0 = b * DIM + d * 128
                nc.sync.dma_start(xT[d], xkvT_d[r0:r0 + 128, :].bitcast(mm_dt))

            # ---- kT = Wk.T @ xT ----
            if b == 0:
                wk = [w_p.tile([128, DI], mm_dt, tag="wk", name=f"wk{_}", bufs=KC6)
                      for _ in range(KC6)]
                for d in range(KC6):
                    nc.sync.dma_start(
                        wk[d], wkv_d[d * 128:(d + 1) * 128, :DI].bitcast(mm_dt))
                wcache["wk"] = wk
            wk = wcache["wk"]
            kT = [kt_p.tile([128, KTOK], mm_dt, tag="kt", name=f"kt{_}")
                  for _ in range(DC8)]
            for d8 in range(DC8):
                for nt in range(2):
                    ps = pstile([128, 320])
                    for k6 in range(KC6):
                        nc.tensor.matmul(
                            ps, wk[k6][:, d8 * 128:(d8 + 1) * 128],
                            xT[k6][:, nt * 320:(nt + 1) * 320],
                            start=(k6 == 0), stop=(k6 == KC6 - 1))
                    nc.vector.tensor_copy(kT[d8][:, nt * 320:(nt + 1) * 320], ps)

            # ---- qT = (Wq*s).T @ xT + bq*s ----
            if b == 0:
                wqs = [w_p.tile([128, DI], mm_dt, tag="wq", name=f"wq{_}", bufs=KC6)
                       for _ in range(KC6)]
                for d in range(KC6):
                    nc.sync.dma_start(
                        wqs[d], wq_d[d * 128:(d + 1) * 128, :].bitcast(mm_dt))
                wcache["wq"] = wqs
            wqs = wcache["wq"]
            qT = [qt_p.tile([128, TOK], mm_dt, tag="qt", name=f"qt{_}")
                  for _ in range(DC8)]
            for d8 in range(DC8):
                ps = pstile([128, 512])
                for k6 in range(KC6):
                    nc.tensor.matmul(
                        ps, wqs[k6][:, d8 * 128:(d8 + 1) * 128],
                        xT[k6][:, W:W + TOK],
                        start=(k6 == 0), stop=(k6 == KC6 - 1))
                nc.scalar.activation(qT[d8], ps, Identity,
                                     bias=bqs_sb[:, d8:d8 + 1])

            # ---- v = xT.T @ Wv (token-major, 65-strided + ones col) ----
            if b == 0:
                wv = [w_p.tile([128, DI], mm_dt, tag="wv", name=f"wv{_}", bufs=KC6)
                      for _ in range(KC6)]
                for d in range(KC6):
                    nc.sync.dma_start(
                        wv[d], wkv_d[d * 128:(d + 1) * 128, DI:].bitcast(mm_dt))
                wcache["wv"] = wv
                load_consts()
            wv = wcache["wv"]
            v_ext = [v_p.tile([128, 16 * 65], bf16, tag="v", name=f"v{_}")
                     for _ in range(NKC)]
            for tt in range(NKC):
                v3 = v_ext[tt].rearrange("p (h c) -> p h c", c=65)
                nc.vector.memset(v3[:, :, 64:65], 1.0)
                for half in range(2):
                    ps = pstile([128, 512])
                    for k6 in range(KC6):
                        nc.tensor.matmul(
                            ps, xT[k6][:, tt * 128:(tt + 1) * 128],
                            wv[k6][:, half * 512:(half + 1) * 512],
                            start=(k6 == 0), stop=(k6 == KC6 - 1))
                    nc.vector.tensor_copy(
                        v3[:, half * 8:(half + 1) * 8, 0:64],
                        ps.rearrange("p (h c) -> p h c", c=64))

            # ---- attention ----
            out_all = [xt_p.tile([128, 1024], f32, tag="xtoa", name=f"oa{_}",
                               bufs=8) for _ in range(NWIN)]
            for hp in range(DC8):
                emem = []
                for h01 in range(2):
                    rows = slice(64 * h01, 64 * h01 + 64)
                    psm = pstile([128, 512])[:4]
                    nc.tensor.matmul(
                        psm, memk_sb[rows, hp * 4:(hp + 1) * 4],
                        qT[hp][rows, :], start=True, stop=True)
                    et = em_p.tile([4, 512], bf16, tag="em", name="em")
                    nc.scalar.activation(et, psm, Exp)
                    emem.append(et)
                exp_tiles = {}
                for kc in range(NKC):
                    qlo = max(0, (kc - 1) * W)
                    qhi = min(TOK, (kc + 1) * W)
                    qw = qhi - qlo
                    off = qlo - (kc - 1) * W
                    bcol = (b * NKC + kc) * 2 * W + off
                    for h01 in range(2):
                        rows = slice(64 * h01, 64 * h01 + 64)
                        ps = pstile([128, 256])[:, :qw]
                        eb = exp_p.tile([128, 256], bf16, tag="expb",
                                        name="expb")[:, :qw]
                        if h01 == 0:
                            nc.tensor.matmul(
                                ps, kT[hp][rows, kc * W:(kc + 1) * W],
                                qT[hp][rows, qlo:qhi], start=True, stop=False)
                            nc.tensor.matmul(
                                ps, identr, bias_sb[:, bcol:bcol + qw],
                                start=False, stop=True)
                            nc.scalar.activation(eb, ps, Exp)
                        else:
                            nc.tensor.matmul(
                                ps, kT[hp][rows, kc * W:(kc + 1) * W],
                                qT[hp][rows, qlo:qhi], start=True, stop=True)
                            et = exp_p.tile([128, 256], f32, tag="expf",
                                            name="expf", bufs=4)[:, :qw]
                            nc.vector.tensor_add(
                                et, ps, bias_sb[:, bcol:bcol + qw].bitcast(f32))
                            nc.scalar.activation(eb, et, Exp)
                        exp_tiles[(h01, kc)] = eb
                allgroups = [(w, h01) for w in range(NWIN) for h01 in range(2)]
                for gi in range(0, 8, 4):
                    groups = allgroups[gi:gi + 4]
                    psvs = {}
                    for w, h01 in groups:
                        hg = 2 * hp + h01
                        psvs[(w, h01)] = pstile([128, 65])
                        nc.tensor.matmul(
                            psvs[(w, h01)], emem[h01][:, w * W:(w + 1) * W],
                            memv_sb[:, hg * 65:(hg + 1) * 65],
                            start=True, stop=False)
                    for w, h01 in groups:
                        hg = 2 * hp + h01
                        pcol = 0 if w == 0 else W
                        nc.tensor.matmul(
                            psvs[(w, h01)], exp_tiles[(h01, w)][:, pcol:pcol + W],
                            v_ext[w].rearrange("p (h c) -> p h c", c=65)[:, hg],
                            start=False, stop=False)
                    for w, h01 in groups:
                        hg = 2 * hp + h01
                        nc.tensor.matmul(
                            psvs[(w, h01)], exp_tiles[(h01, w + 1)][:, 0:W],
                            v_ext[w + 1].rearrange("p (h c) -> p h c", c=65)[:, hg],
                            start=False, stop=True)
                    for w, h01 in groups:
                        hg = 2 * hp + h01
                        psv = psvs[(w, h01)]
                        rc = rc_p.tile([128, 1], f32, tag="rc", name="rc")
                        nc.vector.reciprocal(rc, psv[:, 64:65])
                        nc.vector.tensor_scalar_mul(
                            out_all[w][:, hg * 64:(hg + 1) * 64],
                            psv[:, 0:64], rc)

            # ---- out transpose + final projection ----
            for w in range(NWIN):
                outT = [ot_p.tile([128, 128], mm_dt, tag="ot", name=f"ot{_}")
                        for _ in range(DC8)]
                for d8 in range(DC8):
                    ps = pstile([128, 128])
                    nc.tensor.transpose(
                        ps, out_all[w][:, d8 * 128:(d8 + 1) * 128], ident)
                    nc.vector.tensor_copy(outT[d8], ps)
                ysb = y_p.tile([128, DIM], f32, tag="y", name="y")
                for nn in range(2):
                    ps = pstile([128, 384])
                    for d8 in range(DC8):
                        nc.tensor.matmul(
                            ps, outT[d8], wo_sb[d8][:, nn * 384:(nn + 1) * 384],
                            start=(d8 == 0), stop=(d8 == DC8 - 1))
                    nc.vector.tensor_copy(ysb[:, nn * 384:(nn + 1) * 384], ps)
                nc.sync.dma_start(
                    y_d[b * TOK + w * W:b * TOK + (w + 1) * W, :], ysb)
    nc.compile()
    return nc


def host_prep(x, mask, attn_bias, Wq, bq, Wkv, Wo, memory_kv):
    s = np.float32(DH ** -0.5)
    wq = (np.asarray(Wq, np.float32) * s).astype(np.float32)
    bqs = (np.asarray(bq, np.float32) * s).astype(np.float32).reshape(DC8, 128)
    wkv = np.ascontiguousarray(np.asarray(Wkv, np.float32))
    wo = np.ascontiguousarray(np.asarray(Wo, np.float32))
    x = np.asarray(x, np.float32)
    mask = np.asarray(mask).astype(bool)
    attn_bias = np.asarray(attn_bias, np.float32)
    mk = np.asarray(memory_kv[0], np.float32)
    mv = np.asarray(memory_kv[1], np.float32)

    memk = np.zeros((128, 32), np.float32)
    for hp in range(8):
        memk[0:64, hp * 4:(hp + 1) * 4] = mk[2 * hp].T
        memk[64:128, hp * 4:(hp + 1) * 4] = mk[2 * hp + 1].T
    import ml_dtypes
    memv = np.zeros((4, 16 * 65), np.float32)
    for h in range(H):
        memv[:, h * 65:h * 65 + 64] = mv[h]
        memv[:, h * 65 + 64] = 1.0
    memv = memv.astype(ml_dtypes.bfloat16)

    shared = dict(wq=wq, bqs=bqs, wkv=wkv, wo=wo, memk=memk, memv=memv)
    xT_full = np.ascontiguousarray(x.transpose(0, 2, 1))    # [B, 768, 4096]
    in_maps = []
    for c in range(NCORES):
        q0 = c * TOK
        xkvT = np.zeros((B, DIM, KTOK), np.float32)
        lo = q0 - W
        src_lo = max(lo, 0)
        xkvT[:, :, src_lo - lo:] = xT_full[:, :, src_lo:q0 + TOK]
        biasc = np.full((B, NKC, W, 2 * W), BNEG, np.float32)
        for b in range(B):
            for kc in range(NKC):
                gk = c * NWIN + kc - 1
                if gk < 0:
                    continue
                kr = slice(gk * W, (gk + 1) * W)
                if kc >= 1:
                    qr = slice((c * NWIN + kc - 1) * W, (c * NWIN + kc) * W)
                    biasc[b, kc, :, 0:W] = attn_bias[b, qr, kr].T
                if kc <= NWIN - 1:
                    qr = slice((c * NWIN + kc) * W, (c * NWIN + kc + 1) * W)
                    biasc[b, kc, :, W:2 * W] = attn_bias[b, qr, kr].T
                kmask = mask[b, gk * W:(gk + 1) * W]
                biasc[b, kc, ~kmask, :] = BNEG
        in_maps.append(dict(
            xkvT=np.ascontiguousarray(xkvT.reshape(B * DIM, KTOK)),
            biasc=np.ascontiguousarray(biasc.reshape(B * NKC * W, 2 * W)),
            **shared))
    return in_maps


_CACHE = {}


def kernel(**inputs):
    import sys
    if "/opt/trn_rl_repo" not in sys.path:
        sys.path.insert(0, "/opt/trn_rl_repo")
    from concourse.bass_utils import run_bass_kernel_spmd

    in_maps = host_prep(**inputs)
    if "nc" not in _CACHE:
        _CACHE["nc"] = build_bass()
    nc = _CACHE["nc"]
    res = run_bass_kernel_spmd(nc, in_maps, core_ids=list(range(NCORES)))
    ys = [res.results[c]["y"].reshape(B, TOK, DIM) for c in range(NCORES)]
    return np.concatenate(ys, axis=1)


if __name__ == "__main__":
    import sys
    sys.path.insert(0, "/opt/trn_rl_repo")
    nc = build_bass()
    print("build OK")


# revision 39
# speedup vs baseline: 22247.0427x; 20217.3242x over previous
"""Trainium2 Bass kernel for windowed (sparse) attention with memory KV.

Sequence-sharded across 8 NeuronCores: core c computes output tokens
[c*512, (c+1)*512) for both batches and all heads, with a 1-window (128
token) k/v halo. The full attn_bias is never shipped: only the block-
diagonal and sub-diagonal 128x128 blocks each core needs (pre-transposed,
mask folded in as -inf rows). x is shipped pre-transposed (feature-major)
so no on-device transpose is needed for the input projections.

Device dataflow (per core, per batch):
  qT = (Wq*s).T @ xT + bq*s       [1024, 512]   (feature-major, fp32r)
  kT = Wk.T @ xT                  [1024, 640]   (fp32r)
  v  = xT.T @ Wv                  [640, 1024]   (token-major, +ones col/head)
  per head pair (row-packed K=64 fp32r matmuls):
    simT chunk = kT_chunk.T @ qT  [128 keys, <=256 q]
    exp = Exp(simT + biasT)       (bias add on DVE, Exp on ACT)
    out/sumexp fused: psum[128q, 65] = exp_mem.T@mv_ext + exp_prev.T@vprev_ext
                                       + exp_cur.T@vcur_ext
    out = psum[:, :64] * recip(psum[:, 64])     (per-partition scalar)
  out_all [128q, 1024] -> PE-transpose -> y = outT.T @ Wo -> DMA out
"""

import numpy as np

B, N, DIM = 2, 4096, 768
H, DH = 16, 64
W = 128
DI = H * DH                 # 1024
NEG = -3.4028235e38
BNEG = -1.0e30          # masked-bias value: exp() underflows to 0, but stays
                        # finite under fp32r rounding (0 * -inf would be NaN)
NCORES = 8
TOK = N // NCORES           # 512
NWIN = TOK // W             # 4
KTOK = TOK + W              # 640
NKC = KTOK // W             # 5
KC6 = DIM // 128            # 6 contraction chunks over DIM
DC8 = DI // 128             # 8 chunks over DI

# matmul dtype for the big projections / sim ("float32" or "float32r")
MM_DT_NAME = "float32r"


def build_bass():
    import concourse.mybir as mybir
    import concourse.tile as tile
    from concourse import bacc
    from concourse.masks import make_identity
    from contextlib import ExitStack

    f32 = mybir.dt.float32
    bf16 = mybir.dt.bfloat16
    mm_dt = getattr(mybir.dt, MM_DT_NAME)
    Exp = mybir.ActivationFunctionType.Exp
    Identity = mybir.ActivationFunctionType.Identity
    Copy = mybir.ActivationFunctionType.Copy

    nc = bacc.Bacc("TRN2")

    # xkvT: feature-major x with halo, [B*768, 640]
    xkvT_d = nc.dram_tensor("xkvT", [B * DIM, KTOK], f32, kind="ExternalInput")
    biasc_d = nc.dram_tensor("biasc", [B * NKC * W, 2 * W], f32, kind="ExternalInput")
    wq_d = nc.dram_tensor("wq", [DIM, DI], f32, kind="ExternalInput")
    bqs_d = nc.dram_tensor("bqs", [DC8, 128], f32, kind="ExternalInput")
    wkv_d = nc.dram_tensor("wkv", [DIM, 2 * DI], f32, kind="ExternalInput")
    wo_d = nc.dram_tensor("wo", [DI, DIM], f32, kind="ExternalInput")
    memk_d = nc.dram_tensor("memk", [128, 32], f32, kind="ExternalInput")
    memv_d = nc.dram_tensor("memv", [4, 16 * 65], f32, kind="ExternalInput")
    y_d = nc.dram_tensor("y", [B * TOK, DIM], f32, kind="ExternalOutput")

    with ExitStack() as ctx:
        tc = ctx.enter_context(tile.TileContext(nc))
        # SBUF pools
        const_p = ctx.enter_context(tc.tile_pool(name="const", bufs=1))
        w_p = ctx.enter_context(tc.tile_pool(name="w", bufs=2 * KC6))
        wo_p = ctx.enter_context(tc.tile_pool(name="wo", bufs=DC8))
        xt_p = ctx.enter_context(tc.tile_pool(name="xt", bufs=8))
        kt_p = ctx.enter_context(tc.tile_pool(name="kt", bufs=DC8))
        qt_p = ctx.enter_context(tc.tile_pool(name="qt", bufs=DC8))
        v_p = ctx.enter_context(tc.tile_pool(name="v", bufs=NKC))
        exp_p = ctx.enter_context(tc.tile_pool(name="exp", bufs=12))
        em_p = ctx.enter_context(tc.tile_pool(name="em", bufs=3))
        ot_p = ctx.enter_context(tc.tile_pool(name="ot", bufs=DC8))
        y_p = ctx.enter_context(tc.tile_pool(name="y", bufs=2))
        rc_p = ctx.enter_context(tc.tile_pool(name="rc", bufs=4))
        # single unified PSUM pool: 8 banks cycling
        ps_p = ctx.enter_context(tc.tile_pool(name="ps", bufs=8, space="PSUM"))

        def pstile(shape):
            return ps_p.tile(shape, f32, tag="ps", name="ps",
                             padded_shape=[128, 512])

        ident = const_p.tile([128, 128], f32)
        make_identity(nc, ident)
        identr = const_p.tile([128, 128], mm_dt)
        nc.vector.tensor_copy(identr, ident)

        # const tiles allocated now, DMAs issued later (off the startup path)
        bias_sb = const_p.tile([W, B * NKC * 2 * W], mm_dt)
        memk_sb = const_p.tile([128, 32], mm_dt)
        memv_sb = const_p.tile([4, 16 * 65], f32)
        bqs_sb = const_p.tile([128, DC8], f32)
        wo_sb = [wo_p.tile([128, DIM], mm_dt, tag="wo", name=f"wo{_}")
                 for _ in range(DC8)]

        def load_consts():
            for bb in range(B):
                for kc in range(NKC):
                    col = (bb * NKC + kc) * 2 * W
                    nc.gpsimd.dma_start(
                        bias_sb[:, col:col + 2 * W],
                        biasc_d[(bb * NKC + kc) * W:(bb * NKC + kc + 1) * W,
                                :].bitcast(mm_dt))
            nc.gpsimd.dma_start(memk_sb, memk_d[:, :].bitcast(mm_dt))
            nc.gpsimd.dma_start(memv_sb, memv_d[:, :])

        def load_wo():
            for d in range(DC8):
                nc.gpsimd.dma_start(
                    wo_sb[d], wo_d[d * 128:(d + 1) * 128, :].bitcast(mm_dt))

        nc.gpsimd.dma_start(bqs_sb, bqs_d.rearrange("c p -> p c"))
        wcache = {}
        for b in range(B):
            # ---- xT load (pre-transposed on host) ----
            xT = [xt_p.tile([128, 1024], mm_dt, tag="xtoa", name=f"xt{_}",
                            bufs=8)[:, :KTOK] for _ in range(KC6)]
            for d in range(KC6):
                r0 = b * DIM + d * 128
                nc.sync.dma_start(xT[d], xkvT_d[r0:r0 + 128, :].bitcast(mm_dt))

            # ---- kT = Wk.T @ xT ----
            wk = [w_p.tile([128, DI], mm_dt, tag="wkv", name=f"wkv{_}", bufs=KC6)
                  for _ in range(KC6)]
            for d in range(KC6):
                nc.sync.dma_start(
                    wk[d], wkv_d[d * 128:(d + 1) * 128, :DI].bitcast(mm_dt))
            if b == 0:
                wqs = [w_p.tile([128, DI], mm_dt, tag="wq", name=f"wq{_}", bufs=KC6)
                       for _ in range(KC6)]
                for d in range(KC6):
                    nc.sync.dma_start(
                        wqs[d], wq_d[d * 128:(d + 1) * 128, :].bitcast(mm_dt))
                wcache["wq"] = wqs
            wqs = wcache["wq"]
            wv = [w_p.tile([128, DI], mm_dt, tag="wkv", name=f"wkv{_}", bufs=KC6)
                  for _ in range(KC6)]
            for d in range(KC6):
                nc.sync.dma_start(
                    wv[d], wkv_d[d * 128:(d + 1) * 128, DI:].bitcast(mm_dt))
            if b == 0:
                load_consts()
            kT = [kt_p.tile([128, KTOK], mm_dt, tag="kt", name=f"kt{_}")
                  for _ in range(DC8)]
            for d8 in range(DC8):
                for nt in range(2):
                    ps = pstile([128, 320])
                    for k6 in range(KC6):
                        nc.tensor.matmul(
                            ps, wk[k6][:, d8 * 128:(d8 + 1) * 128],
                            xT[k6][:, nt * 320:(nt + 1) * 320],
                            start=(k6 == 0), stop=(k6 == KC6 - 1))
                    nc.vector.tensor_copy(kT[d8][:, nt * 320:(nt + 1) * 320], ps)

            # ---- qT = (Wq*s).T @ xT + bq*s ----
            qT = [qt_p.tile([128, TOK], mm_dt, tag="qt", name=f"qt{_}")
                  for _ in range(DC8)]
            for d8 in range(DC8):
                ps = pstile([128, 512])
                for k6 in range(KC6):
                    nc.tensor.matmul(
                        ps, wqs[k6][:, d8 * 128:(d8 + 1) * 128],
                        xT[k6][:, W:W + TOK],
                        start=(k6 == 0), stop=(k6 == KC6 - 1))
                nc.scalar.activation(qT[d8], ps, Identity,
                                     bias=bqs_sb[:, d8:d8 + 1])

            # ---- v = xT.T @ Wv (token-major, 65-strided + ones col) ----
            v_ext = [v_p.tile([128, 16 * 65], f32, tag="v", name=f"v{_}")
                     for _ in range(NKC)]
            for tt in range(NKC):
                v3 = v_ext[tt].rearrange("p (h c) -> p h c", c=65)
                nc.vector.memset(v3[:, :, 64:65], 1.0)
                for half in range(2):
                    ps = pstile([128, 512])
                    for k6 in range(KC6):
                        nc.tensor.matmul(
                            ps, xT[k6][:, tt * 128:(tt + 1) * 128],
                            wv[k6][:, half * 512:(half + 1) * 512],
                            start=(k6 == 0), stop=(k6 == KC6 - 1))
                    nc.vector.tensor_copy(
                        v3[:, half * 8:(half + 1) * 8, 0:64],
                        ps.rearrange("p (h c) -> p h c", c=64))

            # ---- attention ----
            out_all = [xt_p.tile([128, 1024], mm_dt, tag="xtoa", name=f"oa{_}",
                               bufs=8) for _ in range(NWIN)]
            for hp in range(DC8):
                if b == 0 and hp == 2:
                    load_wo()
                emem = []
                for h01 in range(2):
                    rows = slice(64 * h01, 64 * h01 + 64)
                    psm = pstile([128, 512])[:4]
                    nc.tensor.matmul(
                        psm, memk_sb[rows, hp * 4:(hp + 1) * 4],
                        qT[hp][rows, :], start=True, stop=True)
                    et = em_p.tile([4, 512], f32, tag="em", name="em")
                    nc.scalar.activation(et, psm, Exp)
                    emem.append(et)
                exp_tiles = {}
                qlo_of = lambda kc: min(max(0, (kc - 1) * W), TOK - 2 * W)
                for kc in range(NKC):
                    qlo = qlo_of(kc)
                    qw = 2 * W
                    bcol = (b * NKC + kc) * 2 * W
                    for h01 in range(2):
                        rows = slice(64 * h01, 64 * h01 + 64)
                        ps = pstile([128, 256])[:, :qw]
                        eb = exp_p.tile([128, 256], f32, tag="expb",
                                        name="expb")[:, :qw]
                        if h01 == 0:
                            nc.tensor.matmul(
                                ps, kT[hp][rows, kc * W:(kc + 1) * W],
                                qT[hp][rows, qlo:qlo + qw], start=True, stop=False)
                            nc.tensor.matmul(
                                ps, identr, bias_sb[:, bcol:bcol + qw],
                                start=False, stop=True)
                            nc.scalar.activation(eb, ps, Exp)
                        else:
                            nc.tensor.matmul(
                                ps, kT[hp][rows, kc * W:(kc + 1) * W],
                                qT[hp][rows, qlo:qlo + qw], start=True, stop=True)
                            et = exp_p.tile([128, 256], f32, tag="expf",
                                            name="expf", bufs=4)[:, :qw]
                            nc.vector.tensor_add(
                                et, ps, bias_sb[:, bcol:bcol + qw].bitcast(f32))
                            nc.scalar.activation(eb, et, Exp)
                        exp_tiles[(h01, kc)] = eb
                allgroups = [(w, h01) for w in range(NWIN) for h01 in range(2)]
                for gi in range(0, 8, 4):
                    groups = allgroups[gi:gi + 4]
                    psvs = {}
                    for w, h01 in groups:
                        hg = 2 * hp + h01
                        psvs[(w, h01)] = pstile([128, 65])
                        nc.tensor.matmul(
                            psvs[(w, h01)], emem[h01][:, w * W:(w + 1) * W],
                            memv_sb[:, hg * 65:(hg + 1) * 65],
                            start=True, stop=False)
                    for w, h01 in groups:
                        hg = 2 * hp + h01
                        pcol = w * W - qlo_of(w)
                        nc.tensor.matmul(
                            psvs[(w, h01)], exp_tiles[(h01, w)][:, pcol:pcol + W],
                            v_ext[w].rearrange("p (h c) -> p h c", c=65)[:, hg],
                            start=False, stop=False)
                    for w, h01 in groups:
                        hg = 2 * hp + h01
                        ccol = w * W - qlo_of(w + 1)
                        nc.tensor.matmul(
                            psvs[(w, h01)],
                            exp_tiles[(h01, w + 1)][:, ccol:ccol + W],
                            v_ext[w + 1].rearrange("p (h c) -> p h c", c=65)[:, hg],
                            start=False, stop=True)
                    for w, h01 in groups:
                        hg = 2 * hp + h01
                        psv = psvs[(w, h01)]
                        rc = rc_p.tile([128, 1], f32, tag="rc", name="rc")
                        nc.vector.reciprocal(rc, psv[:, 64:65])
                        nc.vector.tensor_scalar_mul(
                            out_all[w][:, hg * 64:(hg + 1) * 64],
                            psv[:, 0:64], rc)

            # ---- out transpose + final projection ----
            for w in range(NWIN):
                outT = [ot_p.tile([128, 128], mm_dt, tag="ot", name=f"ot{_}")
                        for _ in range(DC8)]
                for d8 in range(DC8):
                    ps = pstile([128, 128]).bitcast(mm_dt)
                    nc.tensor.transpose(
                        ps, out_all[w][:, d8 * 128:(d8 + 1) * 128], identr)
                    nc.vector.tensor_copy(outT[d8], ps)
                ysb = y_p.tile([128, DIM], f32, tag="y", name="y")
                for nn in range(2):
                    ps = pstile([128, 384])
                    for d8 in range(DC8):
                        nc.tensor.matmul(
                            ps, outT[d8], wo_sb[d8][:, nn * 384:(nn + 1) * 384],
                            start=(d8 == 0), stop=(d8 == DC8 - 1))
                    nc.vector.tensor_copy(ysb[:, nn * 384:(nn + 1) * 384], ps)
                nc.sync.dma_start(
                    y_d[b * TOK + w * W:b * TOK + (w + 1) * W, :], ysb)
    nc.compile()
    return nc


def host_prep(x, mask, attn_bias, Wq, bq, Wkv, Wo, memory_kv):
    s = np.float32(DH ** -0.5)
    wq = (np.asarray(Wq, np.float32) * s).astype(np.float32)
    bqs = (np.asarray(bq, np.float32) * s).astype(np.float32).reshape(DC8, 128)
    wkv = np.ascontiguousarray(np.asarray(Wkv, np.float32))
    wo = np.ascontiguousarray(np.asarray(Wo, np.float32))
    x = np.asarray(x, np.float32)
    mask = np.asarray(mask).astype(bool)
    attn_bias = np.asarray(attn_bias, np.float32)
    mk = np.asarray(memory_kv[0], np.float32)
    mv = np.asarray(memory_kv[1], np.float32)

    memk = np.zeros((128, 32), np.float32)
    for hp in range(8):
        memk[0:64, hp * 4:(hp + 1) * 4] = mk[2 * hp].T
        memk[64:128, hp * 4:(hp + 1) * 4] = mk[2 * hp + 1].T
    memv = np.zeros((4, 16 * 65), np.float32)
    for h in range(H):
        memv[:, h * 65:h * 65 + 64] = mv[h]
        memv[:, h * 65 + 64] = 1.0

    shared = dict(wq=wq, bqs=bqs, wkv=wkv, wo=wo, memk=memk, memv=memv)
    xT_full = np.ascontiguousarray(x.transpose(0, 2, 1))    # [B, 768, 4096]
    in_maps = []
    for c in range(NCORES):
        q0 = c * TOK
        xkvT = np.zeros((B, DIM, KTOK), np.float32)
        lo = q0 - W
        src_lo = max(lo, 0)
        xkvT[:, :, src_lo - lo:] = xT_full[:, :, src_lo:q0 + TOK]
        biasc = np.full((B, NKC, W, 2 * W), BNEG, np.float32)
        for b in range(B):
            for kc in range(NKC):
                gk = c * NWIN + kc - 1
                if gk < 0:
                    continue
                kr = slice(gk * W, (gk + 1) * W)
                qlo = min(max(0, (kc - 1) * W), TOK - 2 * W)
                if kc >= 1:
                    qr = slice((c * NWIN + kc - 1) * W, (c * NWIN + kc) * W)
                    col = (kc - 1) * W - qlo
                    biasc[b, kc, :, col:col + W] = attn_bias[b, qr, kr].T
                if kc <= NWIN - 1:
                    qr = slice((c * NWIN + kc) * W, (c * NWIN + kc + 1) * W)
                    col = kc * W - qlo
                    biasc[b, kc, :, col:col + W] = attn_bias[b, qr, kr].T
                kmask = mask[b, gk * W:(gk + 1) * W]
                biasc[b, kc, ~kmask, :] = BNEG
        in_maps.append(dict(
            xkvT=np.ascontiguousarray(xkvT.reshape(B * DIM, KTOK)),
            biasc=np.ascontiguousarray(biasc.reshape(B * NKC * W, 2 * W)),
            **shared))
    return in_maps


_CACHE = {}


def kernel(**inputs):
    import sys
    if "/opt/trn_rl_repo" not in sys.path:
        sys.path.insert(0, "/opt/trn_rl_repo")
    from concourse.bass_utils import run_bass_kernel_spmd

    in_maps = host_prep(**inputs)
    if "nc" not in _CACHE:
        _CACHE["nc"] = build_bass()
    nc = _CACHE["nc"]
    res = run_bass_kernel_spmd(nc, in_maps, core_ids=list(range(NCORES)))
    ys = [res.results[c]["y"].reshape(B, TOK, DIM) for c in range(NCORES)]
    return np.concatenate(ys, axis=1)


if __name__ == "__main__":
    import sys
    sys.path.insert(0, "/opt/trn_rl_repo")
    nc = build_bass()
    print("build OK")
